# revision 4
# baseline (speedup 1.0000x reference)
"""DPQ joint classification loss on 8 Trainium2 NeuronCores.

reference math (B=4096, D=512, C=10000):
    soft_pred = soft_x @ weight.T ; hard_pred = hard_x @ weight.T
    loss = CE(soft_pred, t) + CE(hard_pred, t)
           + 0.5 * 0.5*(||soft_x - centers[t]||^2 + ||hard_x - centers[t]||^2) / B

Sharding: data-parallel over batch. Core i gets soft rows [i*512,(i+1)*512)
and the matching hard rows, stacked into X = [1024, 512]; weight/centers are
replicated. Each core returns one scalar:
    sum_rows( logsumexp(X @ W^T) - logit_at_target + 0.25*||X - centers[t]||^2 )
and the host computes loss = sum(cores) / B.

Per-core pipeline:
  - PE: fp8e4 DoubleRow GEMM. X is the stationary operand in [128, 2, 128]
    k-pair slices; W^T streams in [128, 2, 512] slices into 512-wide PSUM
    banks, fp32 accumulation. Weight is pre-scaled by 64 on the host so its
    xavier-tiny values use the fp8e4 normal range; the 1/64 descale is
    folded into the exp activation.
  - ACT: exp(psum/64) straight out of PSUM (logits ~N(0, 0.31): no
    max-subtraction needed). Row-sums of exp split between the ACT
    accumulator and DVE reduces to balance engine load.
  - GPSIMD: one indirect-DMA row gather of [weight | centers][targets].
  - DVE: bf16 target-logit (rowsum(x * w_gather)) and quantization
    (rowsum((x - c_gather)^2)) terms, final per-row combine.
  - PE again: cross-partition sum via ones-matmul; DMA scalar out.

DMA triggers cost ~0.65us each on the issuing engine, so transfers are
batched: wt streams as one trigger per class group on the sync queue, xt
on the vector queue, aux loads on gpsimd.
"""

import json

import numpy as np

B_FULL = 4096
D = 512
C = 10000
N_CORES = 8
BS = B_FULL // N_CORES          # 512 rows per core per tensor
B = 2 * BS                      # 1024 stacked rows per core
P = 128
NB = B // P                     # 8 row chunks
NK = D // P                     # 4 contraction chunks of 128
NT = BS // P                    # 4 target chunks
GW = 2048                       # class-group width = 4 PSUM banks
PARAM = 0.5
W_SCALE = 64.0                  # fp8 pre-scale for weight


def _patch_bir_bytes(b: bytes, max_waits: int = 1) -> bytes:
    """Adapt Tile-emitted BIR to this walrus build: it supports only one
    sync-wait per instruction (excess waits move to preceding NoOps) and
    rejects the EVENT_SEMAPHORE_RANGE_CLEAR raw-ISA encoding (replaced by
    per-semaphore write-0 EventSemaphore ops)."""
    d = json.loads(b)
    for f in d["functions"]:
        for blk in f["blocks"]:
            new_insts = []
            for ins in blk["instructions"]:
                if (
                    ins.get("opcode") == "ISA"
                    and ins.get("op_name") == "EVENT_SEMAPHORE_RANGE_CLEAR"
                ):
                    ad = ins.get("ant_dict") or {}
                    for sem_id in range(ad["range_first"], ad["range_last"] + 1):
                        new_insts.append({
                            "name": f"{ins['name']}_clr{sem_id}",
                            "opcode": "EventSemaphore",
                            "engine": ins["engine"],
                            "ins": [],
                            "outs": [],
                            "debug": ins.get("debug"),
                            "sync_info": {
                                "on_wait": [],
                                "on_update": [{
                                    "ant_name": f"semclr_{sem_id}",
                                    "id": sem_id,
                                    "sync_type": "semaphore",
                                    "update_mode": "sem-wr-imm",
                                    "update_value": 0,
                                }],
                            },
                        })
                    continue
                si = ins.get("sync_info")
                waits = (si or {}).get("on_wait") or []
                if len(waits) > max_waits:
                    extra, keep = waits[:-max_waits], waits[-max_waits:]
                    idx = 0
                    while extra:
                        chunk, extra = extra[:max_waits], extra[max_waits:]
                        new_insts.append({
                            "name": f"{ins['name']}_w{idx}",
                            "opcode": "NoOp",
                            "engine": ins["engine"],
                            "ins": [],
                            "outs": [],
                            "debug": ins.get("debug"),
                            "sync_info": {"on_wait": chunk, "on_update": []},
                        })
                        idx += 1
                    si["on_wait"] = keep
                new_insts.append(ins)
            blk["instructions"] = new_insts
    return json.dumps(d).encode()


def _build_bass():
    import concourse.bass as bass
    import concourse.tile as tile
    from concourse import mybir

    f32 = mybir.dt.float32
    bf16 = mybir.dt.bfloat16
    fp8 = mybir.dt.float8e4
    i32 = mybir.dt.int32
    AF = mybir.ActivationFunctionType
    OP = mybir.AluOpType
    DR = mybir.MatmulPerfMode.DoubleRow

    groups = []
    c0 = 0
    while c0 < C:
        groups.append((c0, min(GW, C - c0)))
        c0 += GW
    NG = len(groups)            # 5: 4 x 2048 + 1 x 1808

    nc = bass.Bass()
    # fp8 operands laid out [k_in=128, k_sub, free] for DoubleRow slicing
    xt_d = nc.dram_tensor("xt", [P, NK * B], fp8, kind="ExternalInput")
    wt_d = nc.dram_tensor("wt", [P, NK, C], fp8, kind="ExternalInput")
    # aux operands (bf16 is plenty for the ~1e-3-abs aux terms)
    x_d = nc.dram_tensor("x", [P, NB * D], bf16, kind="ExternalInput")
    wc_d = nc.dram_tensor("wc", [C, 2 * D], bf16, kind="ExternalInput")
    tgt_d = nc.dram_tensor("tgt", [P, NT], i32, kind="ExternalInput")
    out_d = nc.dram_tensor("out", [1, 1], f32, kind="ExternalOutput")

    with tile.TileContext(nc) as tc:
        with (
            tc.tile_pool(name="persist", bufs=1) as persist,
            tc.tile_pool(name="wtp", bufs=2) as wtp,
            tc.tile_pool(name="scratch", bufs=3) as scratch,
            tc.tile_pool(name="esp", bufs=6) as esp,
        ):
            # ---- resident loads ----
            # xt feeds the matmuls: scalar-queue (HWDGE) triggers at the head
            # of the ACT FIFO, parallel to the wt stream on sync, so PE can
            # start ASAP.
            xt3 = persist.tile([P, NK, B], fp8, name="xt3")
            nc.scalar.dma_start(xt3[:, 0:2, :], xt_d[:, 0:2 * B])
            nc.scalar.dma_start(xt3[:, 2:4, :], xt_d[:, 2 * B:4 * B])
            # aux loads on gpsimd SWDGE, off the wt-streaming sync queue.
            tgt_sb = persist.tile([P, NT], i32, name="tgt_sb")
            nc.gpsimd.dma_start(tgt_sb[:, :], tgt_d[:, :])
            x3 = persist.tile([P, NB, D], bf16, name="x3")
            nc.gpsimd.dma_start(x3[:, :, :], x_d[:, :])
            wc_sb = []
            for c in range(NT):
                wc = persist.tile([P, 2 * D], bf16, tag=f"wc{c}", name=f"wc{c}")
                nc.gpsimd.indirect_dma_start(
                    out=wc[:, :], out_offset=None, in_=wc_d[:, :],
                    in_offset=bass.IndirectOffsetOnAxis(ap=tgt_sb[:, c:c + 1], axis=0),
                )
                wc_sb.append(wc)

            # ---- small result tiles ----
            sums = persist.tile([P, NB, NG], f32, name="sums")
            se = persist.tile([P, NB], f32, name="se")
            lse = persist.tile([P, NB], f32, name="lse")
            tcol = persist.tile([P, NB], f32, name="tcol")
            qcol = persist.tile([P, NB], f32, name="qcol")
            aux = persist.tile([P, NB], f32, name="aux")
            ctr2 = persist.tile([P, NB], f32, name="ctr2")
            rowtot = persist.tile([P, 1], f32, name="rowtot")
            ones = persist.tile([P, 1], f32, name="ones")
            nc.vector.memset(ones[:, :], 1.0)

            def aux_block(b):
                # target logit + quantization for row chunk b (DVE, bf16)
                c = b % NT
                pr = scratch.tile([P, D], bf16, tag="pr", name=f"pr{b}")
                nc.vector.tensor_mul(pr[:, :], x3[:, b, :], wc_sb[c][:, :D])
                nc.vector.tensor_reduce(
                    out=tcol[:, b:b + 1], in_=pr[:, :],
                    axis=mybir.AxisListType.X, op=OP.add,
                )
                df = scratch.tile([P, D], bf16, tag="df", name=f"df{b}")
                nc.vector.tensor_sub(df[:, :], x3[:, b, :], wc_sb[c][:, D:])
                sq = scratch.tile([P, D], bf16, tag="sq", name=f"sq{b}")
                nc.vector.tensor_mul(sq[:, :], df[:, :], df[:, :])
                nc.vector.tensor_reduce(
                    out=qcol[:, b:b + 1], in_=sq[:, :],
                    axis=mybir.AxisListType.X, op=OP.add,
                )

            # ---- main GEMM (fp8 DoubleRow) + exp/accumulate ----
            with tc.tile_pool(name="psum", bufs=2, space="PSUM") as psum_pool:
                for g, (c0, cw) in enumerate(groups):
                    wt_g = wtp.tile([P, NK, cw], fp8, tag="wt", name=f"wt{g}")
                    if g == 0:
                        # split so PE can start on the first banks earlier
                        nc.sync.dma_start(
                            wt_g[:, :, :1024], wt_d[:, :, c0:c0 + 1024]
                        )
                        nc.sync.dma_start(
                            wt_g[:, :, 1024:cw], wt_d[:, :, c0 + 1024:c0 + cw]
                        )
                    else:
                        nc.sync.dma_start(wt_g[:, :, :cw], wt_d[:, :, c0:c0 + cw])
                    nbank = (cw + 511) // 512
                    for b in range(NB):
                        ps = psum_pool.tile([P, cw], f32, tag="ps", name=f"ps{g}_{b}")
                        # k-pair outer, banks inner: consecutive matmuls share
                        # the stationary operand (amortizes LDWEIGHTS).
                        for k2 in range(NK // 2):
                            for bank in range(nbank):
                                s0 = bank * 512
                                sw = min(512, cw - s0)
                                nc.tensor.matmul(
                                    ps[:, s0:s0 + sw],
                                    lhsT=xt3[:, 2 * k2:2 * k2 + 2, b * P:(b + 1) * P],
                                    rhs=wt_g[:, 2 * k2:2 * k2 + 2, s0:s0 + sw],
                                    start=(k2 == 0), stop=(k2 == NK // 2 - 1),
                                    perf_mode=DR,
                                )
                        es = esp.tile([P, GW], bf16, tag="es", name=f"es{g}_{b}")
                        idx = g * NB + b
                        if idx % 3 == 0:
                            # ACT accumulator path (costs an extra
                            # ACTIVATION_READ_ACCUMULATOR on the scalar queue)
                            nc.scalar.activation(
                                es[:, :cw], ps[:, :cw], AF.Exp,
                                scale=1.0 / W_SCALE,
                                accum_out=sums[:, b, g:g + 1],
                            )
                        else:
                            # DVE reduce path (scalar stays exp-only)
                            nc.scalar.activation(
                                es[:, :cw], ps[:, :cw], AF.Exp,
                                scale=1.0 / W_SCALE,
                            )
                            nc.vector.tensor_reduce(
                                out=sums[:, b, g:g + 1], in_=es[:, :cw],
                                axis=mybir.AxisListType.X, op=OP.add,
                            )
                        # interleave the aux DVE work mid-run, once the
                        # gathers have long landed
                        if g == 2:
                            aux_block(b)
                    if g == 2:
                        # aux = 0.25*qcol - tcol
                        nc.vector.scalar_tensor_tensor(
                            out=aux[:, :], in0=qcol[:, :], scalar=0.25,
                            in1=tcol[:, :], op0=OP.mult, op1=OP.subtract,
                        )

            # ---- logsumexp + per-row combine ----
            nc.vector.tensor_reduce(
                out=se[:, :], in_=sums[:, :, :],
                axis=mybir.AxisListType.X, op=OP.add,
            )
            nc.scalar.activation(lse[:, :], se[:, :], AF.Ln)
            nc.vector.tensor_add(ctr2[:, :], lse[:, :], aux[:, :])
            nc.vector.tensor_reduce(
                out=rowtot[:, :], in_=ctr2[:, :],
                axis=mybir.AxisListType.X, op=OP.add,
            )

            # ---- cross-partition sum via ones-matmul, write scalar ----
            with tc.tile_pool(name="psum2", bufs=1, space="PSUM") as pp2:
                tot_ps = pp2.tile([1, 1], f32, name="tot_ps")
                nc.tensor.matmul(
                    tot_ps[:, :], lhsT=rowtot[:, :], rhs=ones[:, :],
                    start=True, stop=True,
                )
                tot_sb = persist.tile([1, 1], f32, name="tot_sb")
                nc.vector.tensor_copy(tot_sb[:, :], tot_ps[:, :])
                nc.sync.dma_start(out_d[:, :], tot_sb[:, :])

    orig_to_json = nc.to_json_bytes
    nc.to_json_bytes = lambda: _patch_bir_bytes(orig_to_json())
    return nc


_NC = None


def _get_nc():
    global _NC
    if _NC is None:
        _NC = _build_bass()
    return _NC


def _to_fp8(a: np.ndarray) -> np.ndarray:
    import ml_dtypes

    return np.clip(a, -240.0, 240.0).astype(ml_dtypes.float8_e4m3)


def _make_in_maps(soft_x, hard_x, targets, centers, weight):
    import ml_dtypes

    soft_x = np.asarray(soft_x, np.float32)
    hard_x = np.asarray(hard_x, np.float32)
    targets = np.asarray(targets)
    weight = np.asarray(weight, np.float32)
    centers = np.asarray(centers, np.float32)

    # [k_in=128, k_sub, C] layout: wt8[k_in, ks, c] = 64*W[c, ks*128+k_in]
    wt8 = _to_fp8(
        np.ascontiguousarray(
            (weight.T * W_SCALE).reshape(NK, P, C).transpose(1, 0, 2)
        ).reshape(P, NK * C)
    )
    # [weight | centers] row-gather source, bf16
    wc = np.ascontiguousarray(
        np.concatenate([weight, centers], axis=1).astype(ml_dtypes.bfloat16)
    )

    in_maps = []
    for i in range(N_CORES):
        sl = slice(i * BS, (i + 1) * BS)
        X = np.concatenate([soft_x[sl], hard_x[sl]], axis=0)
        xt8 = _to_fp8(
            np.ascontiguousarray(
                X.T.reshape(NK, P, B).transpose(1, 0, 2)
            ).reshape(P, NK * B)
        )
        xb = np.ascontiguousarray(
            X.reshape(NB, P, D).transpose(1, 0, 2).reshape(P, NB * D)
        ).astype(ml_dtypes.bfloat16)
        tg = np.ascontiguousarray(
            targets[sl].astype(np.int32).reshape(NT, P).T
        )
        in_maps.append(
            {"xt": xt8, "x": xb, "wt": wt8, "wc": wc, "tgt": tg}
        )
    return in_maps


def _run(inputs, trace=False):
    from concourse.bass_utils import run_bass_kernel_spmd

    nc = _get_nc()
    in_maps = _make_in_maps(**inputs)
    res = run_bass_kernel_spmd(
        nc, in_maps, core_ids=list(range(N_CORES)), trace=trace
    )
    total = sum(float(r["out"][0, 0]) for r in res.results)
    return np.float32(total / B_FULL), res


def kernel(soft_x, hard_x, targets, centers, weight):
    loss, _ = _run(
        dict(soft_x=soft_x, hard_x=hard_x, targets=targets,
             centers=centers, weight=weight)
    )
    return loss


# revision 10
# speedup vs baseline: 1.0185x; 1.0185x over previous
"""DPQ joint classification loss on 8 Trainium2 NeuronCores.

reference math (B=4096, D=512, C=10000):
    soft_pred = soft_x @ weight.T ; hard_pred = hard_x @ weight.T
    loss = CE(soft_pred, t) + CE(hard_pred, t)
           + 0.5 * 0.5*(||soft_x - centers[t]||^2 + ||hard_x - centers[t]||^2) / B

Sharding: data-parallel over batch. Core i gets soft rows [i*512,(i+1)*512)
and the matching hard rows, stacked into X = [1024, 512]; weight/centers are
replicated. Each core returns one scalar:
    sum_rows( logsumexp(X @ W^T) - logit_at_target + 0.25*||X - centers[t]||^2 )
and the host computes loss = sum(cores) / B.

Per-core pipeline:
  - PE: fp8e4 DoubleRow GEMM. X is the stationary operand in [128, 2, 128]
    k-pair slices; W^T streams in [128, 2, 512] slices into 512-wide PSUM
    banks, fp32 accumulation. Weight is pre-scaled by 64 on the host so its
    xavier-tiny values use the fp8e4 normal range; the 1/64 descale is
    folded into the exp activation.
  - ACT: exp(psum/64) straight out of PSUM (logits ~N(0, 0.31): no
    max-subtraction needed). Row-sums of exp split between the ACT
    accumulator and DVE reduces to balance engine load.
  - GPSIMD: one indirect-DMA row gather of [weight | centers][targets].
  - DVE: bf16 target-logit (rowsum(x * w_gather)) and quantization
    (rowsum((x - c_gather)^2)) terms, final per-row combine.
  - PE again: cross-partition sum via ones-matmul; DMA scalar out.

DMA triggers cost ~0.65us each on the issuing engine, so transfers are
batched: wt streams as one trigger per class group on the sync queue, xt
on the vector queue, aux loads on gpsimd.
"""

import json

import numpy as np

B_FULL = 4096
D = 512
C = 10000
N_CORES = 8
BS = B_FULL // N_CORES          # 512 rows per core per tensor
B = 2 * BS                      # 1024 stacked rows per core
P = 128
NB = B // P                     # 8 row chunks
NK = D // P                     # 4 contraction chunks of 128
NT = BS // P                    # 4 target chunks
GW = 2048                       # class-group width = 4 PSUM banks
PARAM = 0.5
W_SCALE = 64.0                  # fp8 pre-scale for weight


def _patch_bir_bytes(b: bytes, max_waits: int = 1) -> bytes:
    """Adapt Tile-emitted BIR to this walrus build: it supports only one
    sync-wait per instruction (excess waits move to preceding NoOps) and
    rejects the EVENT_SEMAPHORE_RANGE_CLEAR raw-ISA encoding (replaced by
    per-semaphore write-0 EventSemaphore ops)."""
    d = json.loads(b)
    for f in d["functions"]:
        for blk in f["blocks"]:
            new_insts = []
            for ins in blk["instructions"]:
                if (
                    ins.get("opcode") == "ISA"
                    and ins.get("op_name") == "EVENT_SEMAPHORE_RANGE_CLEAR"
                ):
                    ad = ins.get("ant_dict") or {}
                    for sem_id in range(ad["range_first"], ad["range_last"] + 1):
                        new_insts.append({
                            "name": f"{ins['name']}_clr{sem_id}",
                            "opcode": "EventSemaphore",
                            "engine": ins["engine"],
                            "ins": [],
                            "outs": [],
                            "debug": ins.get("debug"),
                            "sync_info": {
                                "on_wait": [],
                                "on_update": [{
                                    "ant_name": f"semclr_{sem_id}",
                                    "id": sem_id,
                                    "sync_type": "semaphore",
                                    "update_mode": "sem-wr-imm",
                                    "update_value": 0,
                                }],
                            },
                        })
                    continue
                si = ins.get("sync_info")
                waits = (si or {}).get("on_wait") or []
                if len(waits) > max_waits:
                    extra, keep = waits[:-max_waits], waits[-max_waits:]
                    idx = 0
                    while extra:
                        chunk, extra = extra[:max_waits], extra[max_waits:]
                        new_insts.append({
                            "name": f"{ins['name']}_w{idx}",
                            "opcode": "NoOp",
                            "engine": ins["engine"],
                            "ins": [],
                            "outs": [],
                            "debug": ins.get("debug"),
                            "sync_info": {"on_wait": chunk, "on_update": []},
                        })
                        idx += 1
                    si["on_wait"] = keep
                new_insts.append(ins)
            blk["instructions"] = new_insts
    return json.dumps(d).encode()


def _build_bass():
    import concourse.bass as bass
    import concourse.tile as tile
    from concourse import mybir

    f32 = mybir.dt.float32
    bf16 = mybir.dt.bfloat16
    fp8 = mybir.dt.float8e4
    i32 = mybir.dt.int32
    AF = mybir.ActivationFunctionType
    OP = mybir.AluOpType
    DR = mybir.MatmulPerfMode.DoubleRow

    groups = []
    c0 = 0
    while c0 < C:
        groups.append((c0, min(GW, C - c0)))
        c0 += GW
    NG = len(groups)            # 5: 4 x 2048 + 1 x 1808

    nc = bass.Bass()
    # fp8 operands laid out [k_in=128, k_sub, free] for DoubleRow slicing
    xt_d = nc.dram_tensor("xt", [P, NK * B], fp8, kind="ExternalInput")
    wt_d = nc.dram_tensor("wt", [P, NK, C], fp8, kind="ExternalInput")
    # aux operands (bf16 is plenty for the ~1e-3-abs aux terms)
    x_d = nc.dram_tensor("x", [P, NB * D], bf16, kind="ExternalInput")
    wc_d = nc.dram_tensor("wc", [C, 2 * D], bf16, kind="ExternalInput")
    tgt_d = nc.dram_tensor("tgt", [P, NT], i32, kind="ExternalInput")
    out_d = nc.dram_tensor("out", [1, 1], f32, kind="ExternalOutput")

    with tile.TileContext(nc) as tc:
        with (
            nc.allow_low_precision("bf16 partial exp-sums; loss tolerance 2e-2"),
            tc.tile_pool(name="persist", bufs=1) as persist,
            tc.tile_pool(name="wtp", bufs=2) as wtp,
            tc.tile_pool(name="scratch", bufs=3) as scratch,
            tc.tile_pool(name="esp", bufs=6) as esp,
        ):
            # ---- resident loads ----
            # xt feeds the matmuls: head of the gpsimd SWDGE queue, parallel
            # to the wt stream on sync, so PE can start ASAP. (The scalar
            # queue stays pure exp — it is the bottleneck engine.)
            xt3 = persist.tile([P, NK, B], fp8, name="xt3")
            nc.gpsimd.dma_start(xt3[:, 0:2, :], xt_d[:, 0:2 * B])
            nc.gpsimd.dma_start(xt3[:, 2:4, :], xt_d[:, 2 * B:4 * B])
            # aux loads behind xt on gpsimd, off the wt-streaming sync queue.
            tgt_sb = persist.tile([P, NT], i32, name="tgt_sb")
            nc.gpsimd.dma_start(tgt_sb[:, :], tgt_d[:, :])
            x3 = persist.tile([P, NB, D], bf16, name="x3")
            nc.gpsimd.dma_start(x3[:, :, :], x_d[:, :])
            wc_sb = []
            for c in range(NT):
                wc = persist.tile([P, 2 * D], bf16, tag=f"wc{c}", name=f"wc{c}")
                nc.gpsimd.indirect_dma_start(
                    out=wc[:, :], out_offset=None, in_=wc_d[:, :],
                    in_offset=bass.IndirectOffsetOnAxis(ap=tgt_sb[:, c:c + 1], axis=0),
                )
                wc_sb.append(wc)

            # ---- small result tiles ----
            # sums_a: fp32 ACT-accumulator partials; sums_b: bf16 half-group
            # partials from DVE reduces (2B dtypes + 2-elem out aim for the
            # DVE 2x mode)
            sums_a = persist.tile([P, NB, NG], f32, name="sums_a")
            sums_b = persist.tile([P, NB, NG, 2], bf16, name="sums_b")
            sea = persist.tile([P, NB], f32, name="sea")
            seb = persist.tile([P, NB], f32, name="seb")
            se = persist.tile([P, NB], f32, name="se")
            lse = persist.tile([P, NB], f32, name="lse")
            tcol = persist.tile([P, NB], f32, name="tcol")
            qcol = persist.tile([P, NB], f32, name="qcol")
            aux = persist.tile([P, NB], f32, name="aux")
            ctr2 = persist.tile([P, NB], f32, name="ctr2")
            rowtot = persist.tile([P, 1], f32, name="rowtot")
            ones = persist.tile([P, 1], f32, name="ones")
            nc.vector.memset(ones[:, :], 1.0)
            # each (b, g) slot is written by exactly one of the two row-sum
            # paths; zero both so the final reduces see zeros in the other
            nc.vector.memset(sums_a[:, :, :], 0.0)
            nc.vector.memset(sums_b[:, :, :, :], 0.0)

            def aux_block(b):
                # target logit + quantization for row chunk b (DVE, bf16)
                c = b % NT
                pr = scratch.tile([P, D], bf16, tag="pr", name=f"pr{b}")
                nc.vector.tensor_mul(pr[:, :], x3[:, b, :], wc_sb[c][:, :D])
                nc.vector.tensor_reduce(
                    out=tcol[:, b:b + 1], in_=pr[:, :],
                    axis=mybir.AxisListType.X, op=OP.add,
                )
                df = scratch.tile([P, D], bf16, tag="df", name=f"df{b}")
                nc.vector.tensor_sub(df[:, :], x3[:, b, :], wc_sb[c][:, D:])
                sq = scratch.tile([P, D], bf16, tag="sq", name=f"sq{b}")
                nc.vector.tensor_mul(sq[:, :], df[:, :], df[:, :])
                nc.vector.tensor_reduce(
                    out=qcol[:, b:b + 1], in_=sq[:, :],
                    axis=mybir.AxisListType.X, op=OP.add,
                )

            # ---- main GEMM (fp8 DoubleRow) + exp/accumulate ----
            with tc.tile_pool(name="psum", bufs=2, space="PSUM") as psum_pool:
                for g, (c0, cw) in enumerate(groups):
                    wt_g = wtp.tile([P, NK, cw], fp8, tag="wt", name=f"wt{g}")
                    if g == 0:
                        # split so PE can start on the first banks earlier
                        nc.sync.dma_start(
                            wt_g[:, :, :1024], wt_d[:, :, c0:c0 + 1024]
                        )
                        nc.sync.dma_start(
                            wt_g[:, :, 1024:cw], wt_d[:, :, c0 + 1024:c0 + cw]
                        )
                    else:
                        nc.sync.dma_start(wt_g[:, :, :cw], wt_d[:, :, c0:c0 + cw])
                    half = cw // 2
                    for b in range(NB):
                        # [128, 2, 1024] view of 4 PSUM banks: bank index is
                        # 2*h + s//512
                        ps = psum_pool.tile(
                            [P, 2, GW // 2], f32, tag="ps", name=f"ps{g}_{b}"
                        )
                        # k-pair outer, banks inner: consecutive matmuls share
                        # the stationary operand (amortizes LDWEIGHTS).
                        for k2 in range(NK // 2):
                            for h in range(2):
                                for s0 in range(0, half, 512):
                                    sw = min(512, half - s0)
                                    c_lo = h * half + s0
                                    nc.tensor.matmul(
                                        ps[:, h, s0:s0 + sw],
                                        lhsT=xt3[:, 2 * k2:2 * k2 + 2,
                                                 b * P:(b + 1) * P],
                                        rhs=wt_g[:, 2 * k2:2 * k2 + 2,
                                                 c_lo:c_lo + sw],
                                        start=(k2 == 0),
                                        stop=(k2 == NK // 2 - 1),
                                        perf_mode=DR,
                                    )
                        es = esp.tile(
                            [P, 2, GW // 2], bf16, tag="es", name=f"es{g}_{b}"
                        )
                        idx = g * NB + b
                        if idx % 2 == 0:
                            # ACT accumulator path (costs an extra
                            # ACTIVATION_READ_ACCUMULATOR on the scalar queue)
                            nc.scalar.activation(
                                es[:, :, :half], ps[:, :, :half], AF.Exp,
                                scale=1.0 / W_SCALE,
                                accum_out=sums_a[:, b, g:g + 1],
                            )
                        else:
                            # DVE reduce path: bf16 in/out, 2-wide output
                            nc.scalar.activation(
                                es[:, :, :half], ps[:, :, :half], AF.Exp,
                                scale=1.0 / W_SCALE,
                            )
                            nc.vector.tensor_reduce(
                                out=sums_b[:, b, g, :], in_=es[:, :, :half],
                                axis=mybir.AxisListType.X, op=OP.add,
                            )
                        # interleave the aux DVE work mid-run, once the
                        # gathers have long landed
                        if g == 2:
                            aux_block(b)
                    if g == 2:
                        # aux = 0.25*qcol - tcol
                        nc.vector.scalar_tensor_tensor(
                            out=aux[:, :], in0=qcol[:, :], scalar=0.25,
                            in1=tcol[:, :], op0=OP.mult, op1=OP.subtract,
                        )

            # ---- logsumexp + per-row combine ----
            nc.vector.tensor_reduce(
                out=sea[:, :], in_=sums_a[:, :, :],
                axis=mybir.AxisListType.X, op=OP.add,
            )
            nc.vector.tensor_reduce(
                out=seb[:, :], in_=sums_b[:, :, :, :],
                axis=mybir.AxisListType.XY, op=OP.add,
            )
            nc.vector.tensor_add(se[:, :], sea[:, :], seb[:, :])
            nc.scalar.activation(lse[:, :], se[:, :], AF.Ln)
            nc.vector.tensor_add(ctr2[:, :], lse[:, :], aux[:, :])
            nc.vector.tensor_reduce(
                out=rowtot[:, :], in_=ctr2[:, :],
                axis=mybir.AxisListType.X, op=OP.add,
            )

            # ---- cross-partition sum via ones-matmul, write scalar ----
            with tc.tile_pool(name="psum2", bufs=1, space="PSUM") as pp2:
                tot_ps = pp2.tile([1, 1], f32, name="tot_ps")
                nc.tensor.matmul(
                    tot_ps[:, :], lhsT=rowtot[:, :], rhs=ones[:, :],
                    start=True, stop=True,
                )
                tot_sb = persist.tile([1, 1], f32, name="tot_sb")
                nc.vector.tensor_copy(tot_sb[:, :], tot_ps[:, :])
                nc.sync.dma_start(out_d[:, :], tot_sb[:, :])

    orig_to_json = nc.to_json_bytes
    nc.to_json_bytes = lambda: _patch_bir_bytes(orig_to_json())
    return nc


_NC = None


def _get_nc():
    global _NC
    if _NC is None:
        _NC = _build_bass()
    return _NC


def _to_fp8(a: np.ndarray) -> np.ndarray:
    import ml_dtypes

    return np.clip(a, -240.0, 240.0).astype(ml_dtypes.float8_e4m3)


def _make_in_maps(soft_x, hard_x, targets, centers, weight):
    import ml_dtypes

    soft_x = np.asarray(soft_x, np.float32)
    hard_x = np.asarray(hard_x, np.float32)
    targets = np.asarray(targets)
    weight = np.asarray(weight, np.float32)
    centers = np.asarray(centers, np.float32)

    # [k_in=128, k_sub, C] layout: wt8[k_in, ks, c] = 64*W[c, ks*128+k_in]
    wt8 = _to_fp8(
        np.ascontiguousarray(
            (weight.T * W_SCALE).reshape(NK, P, C).transpose(1, 0, 2)
        ).reshape(P, NK * C)
    )
    # [weight | centers] row-gather source, bf16
    wc = np.ascontiguousarray(
        np.concatenate([weight, centers], axis=1).astype(ml_dtypes.bfloat16)
    )

    in_maps = []
    for i in range(N_CORES):
        sl = slice(i * BS, (i + 1) * BS)
        X = np.concatenate([soft_x[sl], hard_x[sl]], axis=0)
        xt8 = _to_fp8(
            np.ascontiguousarray(
                X.T.reshape(NK, P, B).transpose(1, 0, 2)
            ).reshape(P, NK * B)
        )
        xb = np.ascontiguousarray(
            X.reshape(NB, P, D).transpose(1, 0, 2).reshape(P, NB * D)
        ).astype(ml_dtypes.bfloat16)
        tg = np.ascontiguousarray(
            targets[sl].astype(np.int32).reshape(NT, P).T
        )
        in_maps.append(
            {"xt": xt8, "x": xb, "wt": wt8, "wc": wc, "tgt": tg}
        )
    return in_maps


def _run(inputs, trace=False):
    from concourse.bass_utils import run_bass_kernel_spmd

    nc = _get_nc()
    in_maps = _make_in_maps(**inputs)
    res = run_bass_kernel_spmd(
        nc, in_maps, core_ids=list(range(N_CORES)), trace=trace
    )
    total = sum(float(r["out"][0, 0]) for r in res.results)
    return np.float32(total / B_FULL), res


def kernel(soft_x, hard_x, targets, centers, weight):
    loss, _ = _run(
        dict(soft_x=soft_x, hard_x=hard_x, targets=targets,
             centers=centers, weight=weight)
    )
    return loss


# revision 18
# speedup vs baseline: 1.0230x; 1.0044x over previous
"""DPQ joint classification loss on 8 Trainium2 NeuronCores.

reference math (B=4096, D=512, C=10000):
    soft_pred = soft_x @ weight.T ; hard_pred = hard_x @ weight.T
    loss = CE(soft_pred, t) + CE(hard_pred, t)
           + 0.5 * 0.5*(||soft_x - centers[t]||^2 + ||hard_x - centers[t]||^2) / B

Sharding: data-parallel over batch. Core i gets soft rows [i*512,(i+1)*512)
and the matching hard rows, stacked into X = [1024, 512]; weight/centers are
replicated. Each core returns one scalar:
    sum_rows( logsumexp(X @ W^T) - logit_at_target + 0.25*||X - centers[t]||^2 )
and the host computes loss = sum(cores) / B.

Per-core pipeline:
  - PE: fp8e4 DoubleRow GEMM. X is the stationary operand in [128, 2, 128]
    k-pair slices; W^T streams in [128, 2, 512] slices into 512-wide PSUM
    banks, fp32 accumulation. Weight is pre-scaled by 64 on the host so its
    xavier-tiny values use the fp8e4 normal range; the 1/64 descale is
    folded into the exp activation.
  - ACT: exp(psum/64) straight out of PSUM (logits ~N(0, 0.31): no
    max-subtraction needed). Row-sums of exp split between the ACT
    accumulator and DVE reduces to balance engine load.
  - GPSIMD: one indirect-DMA row gather of [weight | centers][targets].
  - DVE: bf16 target-logit (rowsum(x * w_gather)) and quantization
    (rowsum((x - c_gather)^2)) terms, final per-row combine.
  - PE again: cross-partition sum via ones-matmul; DMA scalar out.

DMA triggers cost ~0.65us each on the issuing engine, so transfers are
batched: wt streams as one trigger per class group on the sync queue, xt
on the vector queue, aux loads on gpsimd.
"""

import json

import numpy as np

B_FULL = 4096
D = 512
C = 10000
N_CORES = 8
BS = B_FULL // N_CORES          # 512 rows per core per tensor
B = 2 * BS                      # 1024 stacked rows per core
P = 128
NB = B // P                     # 8 row chunks
NK = D // P                     # 4 contraction chunks of 128
NT = BS // P                    # 4 target chunks
GW = 2048                       # class-group width = 4 PSUM banks
PARAM = 0.5
W_SCALE = 64.0                  # fp8 pre-scale for weight


def _patch_bir_bytes(b: bytes, max_waits: int = 1) -> bytes:
    """Adapt Tile-emitted BIR to this walrus build: it supports only one
    sync-wait per instruction (excess waits move to preceding NoOps) and
    rejects the EVENT_SEMAPHORE_RANGE_CLEAR raw-ISA encoding (replaced by
    per-semaphore write-0 EventSemaphore ops)."""
    d = json.loads(b)
    for f in d["functions"]:
        for blk in f["blocks"]:
            new_insts = []
            for ins in blk["instructions"]:
                if (
                    ins.get("opcode") == "ISA"
                    and ins.get("op_name") == "EVENT_SEMAPHORE_RANGE_CLEAR"
                ):
                    ad = ins.get("ant_dict") or {}
                    for sem_id in range(ad["range_first"], ad["range_last"] + 1):
                        new_insts.append({
                            "name": f"{ins['name']}_clr{sem_id}",
                            "opcode": "EventSemaphore",
                            "engine": ins["engine"],
                            "ins": [],
                            "outs": [],
                            "debug": ins.get("debug"),
                            "sync_info": {
                                "on_wait": [],
                                "on_update": [{
                                    "ant_name": f"semclr_{sem_id}",
                                    "id": sem_id,
                                    "sync_type": "semaphore",
                                    "update_mode": "sem-wr-imm",
                                    "update_value": 0,
                                }],
                            },
                        })
                    continue
                si = ins.get("sync_info")
                waits = (si or {}).get("on_wait") or []
                if len(waits) > max_waits:
                    extra, keep = waits[:-max_waits], waits[-max_waits:]
                    idx = 0
                    while extra:
                        chunk, extra = extra[:max_waits], extra[max_waits:]
                        new_insts.append({
                            "name": f"{ins['name']}_w{idx}",
                            "opcode": "NoOp",
                            "engine": ins["engine"],
                            "ins": [],
                            "outs": [],
                            "debug": ins.get("debug"),
                            "sync_info": {"on_wait": chunk, "on_update": []},
                        })
                        idx += 1
                    si["on_wait"] = keep
                new_insts.append(ins)
            blk["instructions"] = new_insts
    return json.dumps(d).encode()


def _build_bass():
    import concourse.bass as bass
    import concourse.tile as tile
    from concourse import mybir

    f32 = mybir.dt.float32
    bf16 = mybir.dt.bfloat16
    fp8 = mybir.dt.float8e4
    i32 = mybir.dt.int32
    AF = mybir.ActivationFunctionType
    OP = mybir.AluOpType
    DR = mybir.MatmulPerfMode.DoubleRow

    groups = []
    c0 = 0
    while c0 < C:
        groups.append((c0, min(GW, C - c0)))
        c0 += GW
    NG = len(groups)            # 5: 4 x 2048 + 1 x 1808

    nc = bass.Bass()
    # fp8 operands laid out [k_in=128, k_sub, free] for DoubleRow slicing
    xt_d = nc.dram_tensor("xt", [P, NK * B], fp8, kind="ExternalInput")
    wt_d = nc.dram_tensor("wt", [P, NK, C], fp8, kind="ExternalInput")
    # aux operands (bf16 is plenty for the ~1e-3-abs aux terms)
    x_d = nc.dram_tensor("x", [P, NB * D], bf16, kind="ExternalInput")
    wc_d = nc.dram_tensor("wc", [C, 2 * D], bf16, kind="ExternalInput")
    tgt_d = nc.dram_tensor("tgt", [P, NT], i32, kind="ExternalInput")
    out_d = nc.dram_tensor("out", [1, 1], f32, kind="ExternalOutput")

    with tile.TileContext(nc) as tc:
        with (
            nc.allow_low_precision("bf16 partial exp-sums; loss tolerance 2e-2"),
            tc.tile_pool(name="persist", bufs=1) as persist,
            tc.tile_pool(name="wtp", bufs=2) as wtp,
            tc.tile_pool(name="scratch", bufs=3) as scratch,
            tc.tile_pool(name="esp", bufs=6) as esp,
        ):
            # ---- resident loads ----
            # xt feeds the matmuls: head of the gpsimd SWDGE queue, parallel
            # to the wt stream on sync, so PE can start ASAP. (The scalar
            # queue stays pure exp — it is the bottleneck engine.)
            xt3 = persist.tile([P, NK, B], fp8, name="xt3")
            nc.gpsimd.dma_start(xt3[:, 0:2, :], xt_d[:, 0:2 * B])
            nc.gpsimd.dma_start(xt3[:, 2:4, :], xt_d[:, 2 * B:4 * B])
            # aux loads behind xt on gpsimd, off the wt-streaming sync queue.
            tgt_sb = persist.tile([P, NT], i32, name="tgt_sb")
            nc.gpsimd.dma_start(tgt_sb[:, :], tgt_d[:, :])
            x3 = persist.tile([P, NB, D], bf16, name="x3")
            nc.gpsimd.dma_start(x3[:, :, :], x_d[:, :])
            wc_sb = []
            for c in range(NT):
                wc = persist.tile([P, 2 * D], bf16, tag=f"wc{c}", name=f"wc{c}")
                nc.gpsimd.indirect_dma_start(
                    out=wc[:, :], out_offset=None, in_=wc_d[:, :],
                    in_offset=bass.IndirectOffsetOnAxis(ap=tgt_sb[:, c:c + 1], axis=0),
                )
                wc_sb.append(wc)

            # ---- small result tiles ----
            # sums_a: fp32 ACT-accumulator partials; sums_b: bf16 half-group
            # partials from DVE reduces (2B dtypes + 2-elem out aim for the
            # DVE 2x mode)
            sums_a = persist.tile([P, NB, NG], f32, name="sums_a")
            sums_b = persist.tile([P, NB, NG, 2], bf16, name="sums_b")
            sea = persist.tile([P, NB], f32, name="sea")
            seb = persist.tile([P, NB], f32, name="seb")
            se = persist.tile([P, NB], f32, name="se")
            lse = persist.tile([P, NB], f32, name="lse")
            tcol = persist.tile([P, NB], f32, name="tcol")
            qcol = persist.tile([P, NB], f32, name="qcol")
            aux = persist.tile([P, NB], f32, name="aux")
            ctr2 = persist.tile([P, NB], f32, name="ctr2")
            rowtot = persist.tile([P, 1], f32, name="rowtot")
            ones = persist.tile([P, 1], f32, name="ones")
            nc.vector.memset(ones[:, :], 1.0)
            # each (b, g) slot is written by exactly one of the two row-sum
            # paths; zero both so the final reduces see zeros in the other
            nc.vector.memset(sums_a[:, :, :], 0.0)
            nc.vector.memset(sums_b[:, :, :, :], 0.0)

            def aux_block(b):
                # target logit + quantization for row chunk b (DVE, bf16)
                c = b % NT
                pr = scratch.tile([P, D], bf16, tag="pr", name=f"pr{b}")
                nc.vector.tensor_mul(pr[:, :], x3[:, b, :], wc_sb[c][:, :D])
                nc.vector.tensor_reduce(
                    out=tcol[:, b:b + 1], in_=pr[:, :],
                    axis=mybir.AxisListType.X, op=OP.add,
                )
                df = scratch.tile([P, D], bf16, tag="df", name=f"df{b}")
                nc.vector.tensor_sub(df[:, :], x3[:, b, :], wc_sb[c][:, D:])
                sq = scratch.tile([P, D], bf16, tag="sq", name=f"sq{b}")
                nc.vector.tensor_mul(sq[:, :], df[:, :], df[:, :])
                nc.vector.tensor_reduce(
                    out=qcol[:, b:b + 1], in_=sq[:, :],
                    axis=mybir.AxisListType.X, op=OP.add,
                )

            # ---- main GEMM (fp8 DoubleRow) + exp/accumulate ----
            with tc.tile_pool(name="psum", bufs=2, space="PSUM") as psum_pool:
                for g, (c0, cw) in enumerate(groups):
                    wt_g = wtp.tile([P, NK, cw], fp8, tag="wt", name=f"wt{g}")
                    if g == 0:
                        # split so PE can start on the first banks earlier
                        nc.sync.dma_start(
                            wt_g[:, :, :1024], wt_d[:, :, c0:c0 + 1024]
                        )
                        nc.sync.dma_start(
                            wt_g[:, :, 1024:cw], wt_d[:, :, c0 + 1024:c0 + cw]
                        )
                    else:
                        nc.sync.dma_start(wt_g[:, :, :cw], wt_d[:, :, c0:c0 + cw])
                    half = cw // 2
                    for b in range(NB):
                        # [128, 2, 1024] view of 4 PSUM banks: bank index is
                        # 2*h + s//512
                        ps = psum_pool.tile(
                            [P, 2, GW // 2], f32, tag="ps", name=f"ps{g}_{b}"
                        )
                        # k-pair outer, banks inner: consecutive matmuls share
                        # the stationary operand (amortizes LDWEIGHTS).
                        for k2 in range(NK // 2):
                            for h in range(2):
                                for s0 in range(0, half, 512):
                                    sw = min(512, half - s0)
                                    c_lo = h * half + s0
                                    nc.tensor.matmul(
                                        ps[:, h, s0:s0 + sw],
                                        lhsT=xt3[:, 2 * k2:2 * k2 + 2,
                                                 b * P:(b + 1) * P],
                                        rhs=wt_g[:, 2 * k2:2 * k2 + 2,
                                                 c_lo:c_lo + sw],
                                        start=(k2 == 0),
                                        stop=(k2 == NK // 2 - 1),
                                        perf_mode=DR,
                                    )
                        es = esp.tile(
                            [P, 2, GW // 2], bf16, tag="es", name=f"es{g}_{b}"
                        )
                        idx = g * NB + b
                        if idx % 2 == 0:
                            # ACT accumulator path (costs an extra
                            # ACTIVATION_READ_ACCUMULATOR on the scalar queue)
                            nc.scalar.activation(
                                es[:, :, :half], ps[:, :, :half], AF.Exp,
                                scale=1.0 / W_SCALE,
                                accum_out=sums_a[:, b, g:g + 1],
                            )
                        else:
                            # DVE reduce path: bf16 in/out, 2-wide output
                            nc.scalar.activation(
                                es[:, :, :half], ps[:, :, :half], AF.Exp,
                                scale=1.0 / W_SCALE,
                            )
                            nc.vector.tensor_reduce(
                                out=sums_b[:, b, g, :], in_=es[:, :, :half],
                                axis=mybir.AxisListType.X, op=OP.add,
                            )
                        # interleave the aux DVE work mid-run, once the
                        # gathers have long landed
                        if g == 2:
                            aux_block(b)
                    if g == 2:
                        # aux = 0.25*qcol - tcol
                        nc.vector.scalar_tensor_tensor(
                            out=aux[:, :], in0=qcol[:, :], scalar=0.25,
                            in1=tcol[:, :], op0=OP.mult, op1=OP.subtract,
                        )

            # ---- logsumexp + per-row combine ----
            nc.vector.tensor_reduce(
                out=sea[:, :], in_=sums_a[:, :, :],
                axis=mybir.AxisListType.X, op=OP.add,
            )
            nc.vector.tensor_reduce(
                out=seb[:, :], in_=sums_b[:, :, :, :],
                axis=mybir.AxisListType.XY, op=OP.add,
            )
            nc.vector.tensor_add(se[:, :], sea[:, :], seb[:, :])
            nc.scalar.activation(lse[:, :], se[:, :], AF.Ln)
            nc.vector.tensor_add(ctr2[:, :], lse[:, :], aux[:, :])
            nc.vector.tensor_reduce(
                out=rowtot[:, :], in_=ctr2[:, :],
                axis=mybir.AxisListType.X, op=OP.add,
            )

            # ---- cross-partition sum via ones-matmul, write scalar ----
            with tc.tile_pool(name="psum2", bufs=1, space="PSUM") as pp2:
                tot_ps = pp2.tile([1, 1], f32, name="tot_ps")
                nc.tensor.matmul(
                    tot_ps[:, :], lhsT=rowtot[:, :], rhs=ones[:, :],
                    start=True, stop=True,
                )
                tot_sb = persist.tile([1, 1], f32, name="tot_sb")
                nc.vector.tensor_copy(tot_sb[:, :], tot_ps[:, :])
                nc.sync.dma_start(out_d[:, :], tot_sb[:, :])

    orig_to_json = nc.to_json_bytes
    nc.to_json_bytes = lambda: _patch_bir_bytes(orig_to_json())
    return nc


_NC = None


def _get_nc():
    global _NC
    if _NC is None:
        _NC = _build_bass()
    return _NC


def _to_fp8(a: np.ndarray) -> np.ndarray:
    import ml_dtypes

    return np.clip(a, -240.0, 240.0).astype(ml_dtypes.float8_e4m3)


def _make_in_maps(soft_x, hard_x, targets, centers, weight):
    import ml_dtypes

    soft_x = np.asarray(soft_x, np.float32)
    hard_x = np.asarray(hard_x, np.float32)
    targets = np.asarray(targets)
    weight = np.asarray(weight, np.float32)
    centers = np.asarray(centers, np.float32)

    # [k_in=128, k_sub, C] layout: wt8[k_in, ks, c] = 64*W[c, ks*128+k_in]
    wt8 = _to_fp8(
        np.ascontiguousarray(
            (weight.T * W_SCALE).reshape(NK, P, C).transpose(1, 0, 2)
        ).reshape(P, NK * C)
    )
    # [weight | centers] row-gather source, bf16
    wc = np.ascontiguousarray(
        np.concatenate([weight, centers], axis=1).astype(ml_dtypes.bfloat16)
    )

    in_maps = []
    for i in range(N_CORES):
        sl = slice(i * BS, (i + 1) * BS)
        X = np.concatenate([soft_x[sl], hard_x[sl]], axis=0)
        xt8 = _to_fp8(
            np.ascontiguousarray(
                X.T.reshape(NK, P, B).transpose(1, 0, 2)
            ).reshape(P, NK * B)
        )
        xb = np.ascontiguousarray(
            X.reshape(NB, P, D).transpose(1, 0, 2).reshape(P, NB * D)
        ).astype(ml_dtypes.bfloat16)
        tg = np.ascontiguousarray(
            targets[sl].astype(np.int32).reshape(NT, P).T
        )
        in_maps.append(
            {"xt": xt8, "x": xb, "wt": wt8, "wc": wc, "tgt": tg}
        )
    return in_maps


def _run(inputs, trace=False):
    from concourse.bass_utils import run_bass_kernel_spmd

    nc = _get_nc()
    in_maps = _make_in_maps(**inputs)
    res = run_bass_kernel_spmd(
        nc, in_maps, core_ids=list(range(N_CORES)), trace=trace
    )
    total = sum(float(r["out"][0, 0]) for r in res.results)
    return np.float32(total / B_FULL), res


def kernel(soft_x, hard_x, targets, centers, weight):
    loss, _ = _run(
        dict(soft_x=soft_x, hard_x=hard_x, targets=targets,
             centers=centers, weight=weight)
    )
    return loss
